# revision 5
# baseline (speedup 1.0000x reference)
"""Trainium2 Bass kernel for nn_BasicCSRNN (bottom-up tree RNN) — v2.

Strategy (v2 = pruned v1):
- H=256 sharded across 8 cores (32 cols each) -> zero cross-core comms.
- PRUNING: nodes in subtrees under REL nodes never reach the root ->
  dropped entirely. Nodes that are REL or childless have constant
  h = tanh(cb) -> their contribution to the parent is folded on the host
  into a per-parent constant S0 (generalizing v1's m14 trick, which did
  exactly this for the deepest level). Only "active" nodes (alive,
  non-REL, with >=1 child) are computed on device: ~101k slots instead
  of 245k, ~91k scatter children instead of 245k.
- Per level: psum = sel-matmul scatter of child m blocks (fp8 0/1
  selection matrices, sources sorted by parent window);
  h = tanh(psum + S0cb) where S0cb = c + b + S0 uploaded per level;
  m = h * scale.
"""
import os
import sys

sys.path.insert(0, "/opt/trn_rl_repo")
import numpy as np

D, W = 16, 16384
N = 1 + (D - 1) * W
H, I, E = 256, 256, 16
NCORES = 8
HS = H // NCORES  # 32
WS = 128          # dst window size (slots) = one 128-slot block

_cache = {}
LAST_EXEC_NS = None


def _install_profhook():
    import types
    try:
        from antenv import axon_hooks  # noqa: F401
        return
    except ImportError:
        pass
    import antenv
    mod = types.ModuleType("antenv.axon_hooks")
    _hook = [None]
    mod.set_axon_ntff_profile_hook = lambda h: _hook.__setitem__(0, h)
    mod.get_axon_ntff_profile_hook = lambda: _hook[0]
    sys.modules["antenv.axon_hooks"] = mod
    antenv.axon_hooks = mod
    from trn_agent_boot.trn_boot import _ntff_profile_via_ctypes
    mod.set_axon_ntff_profile_hook(
        _ntff_profile_via_ctypes("/opt/axon/libaxon_pjrt.so"))
    import concourse.bass_utils as bu
    bu.upload_artifacts = lambda tmpdir: "local://" + str(tmpdir)


def _analyze(parent, levels, is_rel):
    """Graph analysis: alive/active sets + const-children lists."""
    lv = [np.asarray(levels[d], np.int64) for d in range(D - 1)]
    alive = np.zeros(N, bool)
    alive[0] = True
    alive[lv[0]] = True
    for d in range(1, D - 1):
        p = parent[lv[d]]
        alive[lv[d]] = alive[p] & ~is_rel[p]
    cnt = np.zeros(N, np.int64)
    for d in range(D - 1):
        np.add.at(cnt, parent[lv[d]], 1)
    active = alive & ~is_rel & (cnt > 0)
    active[0] = False
    # const children: alive, not active (REL or childless), with alive
    # non-REL parent -> contribute tanhcb*We'[etype] to parent's S0.
    const_nodes = []
    for d in range(D - 1):
        nodes = lv[d]
        c = nodes[alive[nodes] & ~active[nodes]]
        p = parent[c]
        keep = alive[p] & ~is_rel[p]
        const_nodes.append(c[keep])
    return lv, alive, active, const_nodes


def _build_structure(parent, levels, is_rel):
    lv, alive, active, const_nodes = _analyze(parent, levels, is_rel)
    act_lv = [lv[d][active[lv[d]]] for d in range(D - 1)]
    # level 14 must be all-const
    assert len(act_lv[D - 2]) == 0
    NL = D - 2  # deepest level with active nodes is NL-1 = 13

    slotpos = np.full(N, -1, np.int64)
    layouts = []
    for d in range(NL):
        nodes = act_lv[d]
        if d == 0:
            snodes = np.sort(nodes)
            nslot = ((len(snodes) + 127) // 128) * 128
            slot_node = np.full(nslot, -1, np.int64)
            slot_node[:len(snodes)] = snodes
            slotpos[snodes] = np.arange(len(snodes))
            layouts.append({"slot_node": slot_node, "nslot": nslot})
            continue
        # whole-window bin-packing: a window's children never straddle a
        # block (oversized windows split into dedicated 128-chunks), so
        # each (block, window) incidence is exactly one sel entry.
        pslot = slotpos[parent[nodes]]
        order = np.argsort(pslot, kind="stable")
        snodes_s = nodes[order]
        pw_s = pslot[order] // WS
        nw_prev = layouts[d - 1]["nslot"] // WS
        slot_list = []
        cur = 0
        for t in range(nw_prev):
            ch = snodes_s[pw_s == t]
            g = len(ch)
            if g == 0:
                continue
            i = 0
            while g > 128:
                if cur:
                    slot_list.append(np.full(128 - cur, -1, np.int64))
                    cur = 0
                slot_list.append(ch[i:i + 128])
                i += 128
                g -= 128
            if g > 128 - cur and cur > 0:
                slot_list.append(np.full(128 - cur, -1, np.int64))
                cur = 0
            slot_list.append(ch[i:])
            cur = (cur + g) % 128
        if cur:
            slot_list.append(np.full(128 - cur, -1, np.int64))
        slot_node = np.concatenate(slot_list)
        nslot = len(slot_node)
        assert nslot % 128 == 0
        real = slot_node >= 0
        slotpos[slot_node[real]] = np.nonzero(real)[0]
        layouts.append({"slot_node": slot_node, "nslot": nslot})

    # per-transition entries + fp8 sel streams (core-independent)
    import ml_dtypes
    transitions = []
    for d in range(1, NL):
        L = layouts[d]
        slot_node = L["slot_node"]
        nb = L["nslot"] // 128
        nw_prev = layouts[d - 1]["nslot"] // WS
        sp = np.full(L["nslot"], -1, np.int64)
        real = slot_node >= 0
        sp[real] = slotpos[parent[slot_node[real]]]
        win_of = np.where(sp >= 0, sp // WS, -1).reshape(nb, 128)
        entries = []
        for s in range(nb):
            for t in np.unique(win_of[s]):
                if t >= 0:
                    entries.append((s, int(t)))
        have = {t for _, t in entries}
        empties = [t for t in range(nw_prev) if t not in have]
        entries.sort(key=lambda e: (e[1], e[0]))
        ne = len(entries)
        sel = np.zeros((128, ne * WS), ml_dtypes.float8_e4m3)
        spb = sp.reshape(nb, 128)
        for e, (s, t) in enumerate(entries):
            rows = spb[s]
            k = np.nonzero((rows >= t * WS) & (rows < (t + 1) * WS))[0]
            sel[k, e * WS + (rows[k] - t * WS)] = 1.0
        transitions.append({"entries": entries, "sel": sel, "nb": nb,
                            "nw_prev": nw_prev, "d": d, "empties": empties})
    return layouts, transitions, act_lv, const_nodes, slotpos


def _compile(layouts, transitions):
    import concourse.bass as bass
    import concourse.bacc as bacc
    import concourse.mybir as mybir
    import concourse.tile as tile

    f32 = mybir.dt.float32
    f16 = mybir.dt.float16
    f8 = mybir.dt.float8e4

    NL = len(layouts)
    nc = bacc.Bacc("TRN2", target_bir_lowering=False, debug=False,
                   num_devices=NCORES)
    nb = [L["nslot"] // 128 for L in layouts]
    ng = [(b + 15) // 16 for b in nb]

    sel_in = {}
    for tr in transitions:
        d = tr["d"]
        ne = len(tr["entries"])
        sel_in[d] = nc.dram_tensor(f"sel{d}", [128, ne * WS], f8,
                                   kind="ExternalInput")
    scale_in = {d: nc.dram_tensor(f"scale{d}", [128, nb[d] * HS],
                                  f8 if d >= 2 else f16,
                                  kind="ExternalInput")
                for d in range(NL)}
    s0cb_in = {d: nc.dram_tensor(f"s0cb{d}", [128, nb[d] * HS],
                                 f8 if d >= 2 else f16,
                                 kind="ExternalInput")
               for d in range(NL)}
    ones_in = nc.dram_tensor("ones", [128, 1], f32, kind="ExternalInput")
    root_out = nc.dram_tensor("root", [1, HS], f32, kind="ExternalOutput")

    SELCH = 32

    with tile.TileContext(nc) as tc:
        with tc.tile_pool(name="const", bufs=1) as cpool, \
             tc.tile_pool(name="work", bufs=3) as pool, \
             tc.tile_pool(name="selp", bufs=14) as selpool, \
             tc.tile_pool(name="psum", bufs=7, space="PSUM") as psum_pool:
            ones_t = cpool.tile([128, 1], f32, tag="ones")
            nc.sync.dma_start(out=ones_t[:], in_=ones_in[:])

            # deepest active level (NL-1): h = tanh(s0cb), m = h*scale
            ddeep = NL - 1
            s0_deep = pool.tile([128, nb[ddeep] * HS],
                                f8 if ddeep >= 2 else f16, tag="s0d")
            nc.sync.dma_start(out=s0_deep[:], in_=s0cb_in[ddeep][:])
            sc_deep = pool.tile([128, nb[ddeep] * HS],
                                f8 if ddeep >= 2 else f16, tag="scd")
            nc.sync.dma_start(out=sc_deep[:], in_=scale_in[ddeep][:])
            hh_deep = pool.tile([128, nb[ddeep] * HS], f16, tag="hhd")
            nc.scalar.activation(out=hh_deep[:], in_=s0_deep[:],
                                 func=mybir.ActivationFunctionType.Tanh)
            m_deep = pool.tile([128, nb[ddeep] * HS],
                               f8 if ddeep >= 4 else f16, tag="md")
            nc.vector.tensor_tensor(out=m_deep[:], in0=hh_deep[:],
                                    in1=sc_deep[:], op=mybir.AluOpType.mult)
            m_chunks = [(m_deep, 512 * g) for g in range(ng[ddeep])]

            for tr in reversed(transitions):   # d = NL-1 .. 1
                d = tr["d"]
                dd = d - 1
                entries = tr["entries"]
                empties = set(tr["empties"])
                ne = len(entries)
                nwd = nb[dd]
                sel_tiles = []
                for c in range(0, ne, SELCH):
                    hi = min(c + SELCH, ne)
                    st = selpool.tile([128, SELCH * WS], f8, tag="sel")
                    nc.sync.dma_start(out=st[:, :(hi - c) * WS],
                                      in_=sel_in[d][:, c * WS:hi * WS])
                    sel_tiles.append(st)
                sc_t = pool.tile([128, nb[dd] * HS], f8 if dd >= 2 else f16,
                                 tag="scale")
                nc.sync.dma_start(out=sc_t[:], in_=scale_in[dd][:])
                s0_t = pool.tile([128, nb[dd] * HS],
                                 f8 if dd >= 2 else f16, tag="s0cb")
                nc.sync.dma_start(out=s0_t[:], in_=s0cb_in[dd][:])

                new_chunks = []
                eidx = 0
                for g in range(ng[dd]):
                    wlo, whi = g * 16, min((g + 1) * 16, nwd)
                    nwin = whi - wlo
                    ps = psum_pool.tile([128, 512], f32, tag="ps")
                    covered = set()
                    while eidx < ne and entries[eidx][1] < whi:
                        s, t = entries[eidx]
                        first = t not in covered
                        covered.add(t)
                        last = (eidx + 1 == ne) or (entries[eidx + 1][1] != t)
                        st = sel_tiles[eidx // SELCH]
                        off = (eidx % SELCH) * WS
                        mt, mo = m_chunks[s // 16]
                        nc.tensor.matmul(
                            out=ps[:, (t % 16) * HS:(t % 16 + 1) * HS],
                            lhsT=st[:, off:off + WS],
                            rhs=mt[:, mo + (s % 16) * HS:mo + (s % 16 + 1) * HS],
                            start=first, stop=last)
                        eidx += 1
                    for t in range(wlo, whi):
                        if t in empties:
                            nc.vector.memset(
                                ps[:, (t % 16) * HS:(t % 16 + 1) * HS], 0)
                    hp = pool.tile([128, nwin * HS], f32, tag=f"hp{g % 4}")
                    nc.vector.tensor_tensor(
                        out=hp[:], in0=ps[:, :nwin * HS],
                        in1=s0_t[:, g * 512:g * 512 + nwin * HS],
                        op=mybir.AluOpType.add)
                    hh = pool.tile([128, nwin * HS], f16, tag=f"hh{g % 4}")
                    nc.scalar.activation(out=hh[:], in_=hp[:],
                                         func=mybir.ActivationFunctionType.Tanh)
                    mc = pool.tile([128, nwin * HS], f8 if dd >= 4 else f16,
                                   tag=f"mc{g % 8}")
                    nc.vector.tensor_tensor(
                        out=mc[:], in0=hh[:],
                        in1=sc_t[:, g * 512:g * 512 + nwin * HS],
                        op=mybir.AluOpType.mult)
                    new_chunks.append((mc, 0))
                    if dd == 0:
                        rcg = pool.tile([128, HS], f32, tag=f"rcg{g % 2}",
                                        name=f"rcg_{g}")
                        apm = mc[:]
                        nc.vector.tensor_reduce(
                            out=rcg[:],
                            in_=bass.AP(apm.tensor, apm.offset,
                                        [[apm.ap[0][0], 128], [1, HS],
                                         [HS, nwin]]),
                            axis=mybir.AxisListType.X,
                            op=mybir.AluOpType.add)
                        if g == 0:
                            red0_t = pool.tile([128, HS], f32, tag="red0",
                                               name="red0_t")
                            nc.vector.tensor_copy(out=red0_t[:], in_=rcg[:])
                        else:
                            nc.vector.tensor_tensor(
                                out=red0_t[:], in0=red0_t[:], in1=rcg[:],
                                op=mybir.AluOpType.add)
                m_chunks = new_chunks

            # ---- root reduce: sum all level-0 m rows ----
            rps = psum_pool.tile([128, HS], f32, tag="ps")
            nc.tensor.matmul(out=rps[0:1, :], lhsT=ones_t[:], rhs=red0_t[:],
                             start=True, stop=True)
            rout = pool.tile([1, HS], f32, tag="ro")
            nc.vector.tensor_copy(out=rout[:], in_=rps[0:1, :])
            nc.sync.dma_start(out=root_out[:], in_=rout[:])

    nc.finalize()
    return nc


def kernel(embedding, Wx, We, b, parent, etype, levels, is_rel):
    from concourse.bass_utils import run_bass_kernel_spmd
    import ml_dtypes

    embedding = np.asarray(embedding, np.float32)
    Wx = np.asarray(Wx, np.float32)
    We = np.asarray(We, np.float32)
    b = np.asarray(b, np.float32)
    parent = np.asarray(parent, np.int64)
    etype = np.asarray(etype, np.int64)
    levels_np = np.asarray(levels, np.int64)
    is_rel = np.asarray(is_rel, bool)

    import hashlib
    key = hashlib.sha1(b"".join([parent.tobytes(), is_rel.tobytes(),
                                 levels_np.tobytes()])).hexdigest()
    if key not in _cache:
        layouts, transitions, act_lv, const_nodes, slotpos = \
            _build_structure(parent, levels_np, is_rel)
        nc = _compile(layouts, transitions)
        _cache[key] = (layouts, transitions, act_lv, const_nodes, slotpos, nc)
    layouts, transitions, act_lv, const_nodes, slotpos, nc = _cache[key]
    NL = len(layouts)

    # ---- numeric host inputs ----
    c = embedding @ Wx                       # [H]
    cb_full = c + b[0]                       # [H]
    tanhcb = np.tanh(cb_full)
    WeT = We[:, 0, :]                        # [E, H]
    cmT = tanhcb[None, :] * WeT              # [E, H] const-child m by etype

    # S0 per parent: sum of const-children contributions
    cnt_pe = np.zeros((N, E), np.float32)
    for d in range(NL + 1):
        cc = const_nodes[d] if d < len(const_nodes) else np.empty(0, np.int64)
        if len(cc):
            np.add.at(cnt_pe, (parent[cc], etype[cc]), 1.0)
    S0 = cnt_pe @ cmT                        # [N, H]

    nbs = [L["nslot"] // 128 for L in layouts]
    in_maps = []
    for core in range(NCORES):
        cs = slice(core * HS, (core + 1) * HS)
        m = {"ones": np.ones((128, 1), np.float32)}
        for tr in transitions:
            m[f"sel{tr['d']}"] = tr["sel"]
        for d in range(NL):
            L = layouts[d]
            sn = L["slot_node"]
            nb = nbs[d]
            real = sn >= 0
            sc = np.zeros((L["nslot"], HS), np.float32)
            sc[real] = WeT[etype[sn[real]]][:, cs]
            scr = sc.reshape(nb, 128, HS).transpose(1, 0, 2).reshape(
                128, nb * HS)
            m[f"scale{d}"] = scr.astype(
                ml_dtypes.float8_e4m3 if d >= 2 else np.float16)
            s0 = np.zeros((L["nslot"], HS), np.float32)
            s0[:] = cb_full[cs][None, :]
            s0[real] += S0[sn[real]][:, cs]
            s0r = s0.reshape(nb, 128, HS).transpose(1, 0, 2).reshape(
                128, nb * HS)
            m[f"s0cb{d}"] = s0r.astype(
                ml_dtypes.float8_e4m3 if d >= 2 else np.float16)
        in_maps.append(m)

    trace = bool(os.environ.get("CSRNN_TRACE"))
    kw = {}
    if trace:
        import tempfile
        _install_profhook()
        kw = {"trace": True, "tmpdir": tempfile.mkdtemp(prefix="csrnn_")}
    res = run_bass_kernel_spmd(nc, in_maps, list(range(NCORES)), **kw)
    global LAST_EXEC_NS
    LAST_EXEC_NS = res.exec_time_ns
    acc0 = np.concatenate([res.results[core]["root"][0]
                           for core in range(NCORES)])
    root_hidden = acc0 + S0[0]               # device active-sum + const-sum
    if is_rel[0]:
        root_hidden = np.zeros(H, np.float32)
    out = np.tanh(c + root_hidden + b[0])
    return out[None, :].astype(np.float32)
